# revision 15
# baseline (speedup 1.0000x reference)
"""JPEG compression roundtrip kernel for Trainium2 (8 NeuronCores, batch-parallel).

Self-contained: builds constants, shards batch 32 -> 8 cores x 4 images,
runs a Bass/Tile kernel per core, gathers full output.

Pipeline per image (512x512x3 f32 in [0,1)), all matmuls fp16 (1 cyc/row):
  S1  Act: fp16 <- 255*x + 1535.5 (write rounds to int); DVE: -1536 -> u8 exact
  p1  (stat=u8 fp16 chunks, mov=color-scaled DCT fp16)  -> M1 = (A@{Y,Cb,Cr})^T
  p2  (stat=DCT const fp16, mov=M1 fp16)                -> coef PSUM f32
  q   y = coef*(1/t) [DVE]; q = rne(y) -> fp16 exact [DVE]; deq = q*t fp16 [DVE 2x]
  p3  (stat=deq fp16, mov=IDCT const fp16)              -> M3 PSUM -> fp16
  p4  (stat=IDCT+color consts fp16, mov=M3 fp16)        -> R,G,B planes PSUM f32
  S5  single op: float->uint8 convert (HW does RNE+saturate) -> interleaved u8 out
Host: out = u8.astype(f32) / 255.

4:2:0 chroma down/upsample folded into chroma DCT matrices (E = D@P, V = 2E^T);
+-128 level shifts cancel because the DC quant step divides the DC shift.
Dequantized coefficients q*t are integers <= 2046: exact in fp16.
"""
import numpy as np

from concourse import bacc, bass, mybir, tile
from concourse.bass_utils import run_bass_kernel_spmd

F = np.float32
C_RNE = float(np.float32(12582912.0))  # 1.5 * 2**23
N_CORES = 8
B_PER_CORE = 4
DT = mybir.dt.float32
DT_H = mybir.dt.float16
DT_U8 = mybir.dt.uint8
QUALITY = 95

_LUMA = np.array([
    [16, 11, 10, 16, 24, 40, 51, 61],
    [12, 12, 14, 19, 26, 58, 60, 55],
    [14, 13, 16, 24, 40, 57, 69, 56],
    [14, 17, 22, 29, 51, 87, 80, 62],
    [18, 22, 37, 56, 68, 109, 103, 77],
    [24, 35, 55, 64, 81, 104, 113, 92],
    [49, 64, 78, 87, 103, 121, 120, 101],
    [72, 92, 95, 98, 112, 100, 103, 99]], dtype=F)
_CHROMA = np.array([
    [17, 18, 24, 47, 99, 99, 99, 99],
    [18, 21, 26, 66, 99, 99, 99, 99],
    [24, 26, 56, 99, 99, 99, 99, 99],
    [47, 66, 99, 99, 99, 99, 99, 99],
    [99, 99, 99, 99, 99, 99, 99, 99],
    [99, 99, 99, 99, 99, 99, 99, 99],
    [99, 99, 99, 99, 99, 99, 99, 99],
    [99, 99, 99, 99, 99, 99, 99, 99]], dtype=F)


def _qtable(base, quality):
    scale = 5000.0 / quality if quality < 50 else 200.0 - 2.0 * quality
    return np.clip(np.floor((base * scale + 50.0) / 100.0), 1.0, 255.0).astype(F)


def build_consts():
    k = np.arange(8)
    D = np.sqrt(2.0 / 8.0) * np.cos((2 * k[None, :] + 1) * k[:, None] * np.pi / 16.0)
    D[0, :] /= np.sqrt(2.0)
    D = D.astype(F)
    P = np.zeros((8, 16), F)
    for i in range(8):
        P[i, 2 * i] = 0.5
        P[i, 2 * i + 1] = 0.5
    E = (D @ P).astype(F)
    V = (2.0 * E.T).astype(F)
    QL = _qtable(_LUMA, QUALITY)
    QC = _qtable(_CHROMA, QUALITY)
    I16 = np.eye(16, dtype=F)
    I8 = np.eye(8, dtype=F)
    cY = np.array([0.299, 0.587, 0.114], F)
    cCb = np.array([-0.168736, -0.331264, 0.5], F)
    cCr = np.array([0.5, -0.418688, -0.081312], F)

    c = {}
    mv_fy = np.kron(I16, D.T).astype(F)
    mv_fc = np.kron(I8, E.T).astype(F)
    for ch in range(3):
        mv = np.ascontiguousarray(np.concatenate(
            [cY[ch] * mv_fy, cCb[ch] * mv_fc, cCr[ch] * mv_fc], axis=1).astype(F))
        c[f"mvp1_{ch}"] = mv.astype(np.float16)
    c["sp2y"] = mv_fy.astype(np.float16)
    c["sp2c"] = np.ascontiguousarray(
        np.pad(mv_fc, ((0, 0), (0, 64)))).astype(np.float16)
    c["mvp3y"] = np.kron(I16, D).astype(np.float16)
    c["sp4y"] = np.kron(I16, D).astype(np.float16)
    sp4c = np.kron(I16, V).T.astype(F)  # [128 fhc, 256 h]
    wR_cr, wG_cb, wG_cr, wB_cb = 1.402, -0.344136, -0.714136, 1.772
    for h in range(2):
        sl = np.ascontiguousarray(sp4c[:, 128 * h:128 * (h + 1)])
        c[f"sp4c_h{h}_rcr"] = (F(wR_cr) * sl).astype(np.float16)
        c[f"sp4c_h{h}_gcb"] = (F(wG_cb) * sl).astype(np.float16)
        c[f"sp4c_h{h}_gcr"] = (F(wG_cr) * sl).astype(np.float16)
        c[f"sp4c_h{h}_bcb"] = (F(wB_cb) * sl).astype(np.float16)
    tY = np.empty((128, 512), F)
    pp, ff = np.meshgrid(np.arange(128), np.arange(512), indexing="ij")
    tY[:] = QL[ff % 8, pp % 8]
    tC = np.empty((128, 256), F)
    pp, ff = np.meshgrid(np.arange(128), np.arange(256), indexing="ij")
    tC[:] = QC[ff % 8, pp % 8]
    c["taby"] = tY.reshape(128, 4, 128).astype(np.float16)
    c["rtaby"] = (1.0 / tY).astype(F).reshape(128, 4, 128).copy()
    c["tabc"] = tC.reshape(128, 2, 128).astype(np.float16)
    c["rtabc"] = (1.0 / tC).astype(F).reshape(128, 2, 128).copy()
    return c


# fp16 consts packed into one [128, 3328] tensor; fp32 (recip tables) into
# one [128, 768]: two DMAs instead of twenty (HWDGE is exclusive, 625ns each).
H16A_ORDER = ["mvp1_0", "mvp1_1", "mvp1_2"]
H16B_ORDER = ["sp2y", "sp2c", "mvp3y",
              "sp4y",
              "sp4c_h0_rcr", "sp4c_h0_gcb", "sp4c_h0_gcr", "sp4c_h0_bcb",
              "sp4c_h1_rcr", "sp4c_h1_gcb", "sp4c_h1_gcr", "sp4c_h1_bcb",
              "taby", "tabc"]
H16_ORDER = H16A_ORDER + H16B_ORDER
F32_ORDER = ["rtaby", "rtabc"]

CONST_SHAPES = {
    "mvp1_0": (128, 256), "mvp1_1": (128, 256), "mvp1_2": (128, 256),
    "sp2y": (128, 128), "sp2c": (128, 128),
    "mvp3y": (128, 128),
    "sp4y": (128, 128),
    "sp4c_h0_rcr": (128, 128), "sp4c_h0_gcb": (128, 128),
    "sp4c_h0_gcr": (128, 128), "sp4c_h0_bcb": (128, 128),
    "sp4c_h1_rcr": (128, 128), "sp4c_h1_gcb": (128, 128),
    "sp4c_h1_gcr": (128, 128), "sp4c_h1_bcb": (128, 128),
    "taby": (128, 4, 128), "rtaby": (128, 4, 128),
    "tabc": (128, 2, 128), "rtabc": (128, 2, 128),
}


def _ncols(shape):
    n = 1
    for s in shape[1:]:
        n *= s
    return n


def pack_consts(c):
    import numpy as _np
    h16 = _np.concatenate(
        [c[k].reshape(128, -1) for k in H16_ORDER], axis=1).astype(_np.float16)
    f32 = _np.concatenate(
        [c[k].reshape(128, -1) for k in F32_ORDER], axis=1).astype(F)
    return {"ch16": _np.ascontiguousarray(h16), "cf32": _np.ascontiguousarray(f32)}


H16A_TOTAL = sum(_ncols(CONST_SHAPES[k]) for k in H16A_ORDER)
H16_TOTAL = sum(_ncols(CONST_SHAPES[k]) for k in H16_ORDER)
F32_TOTAL = sum(_ncols(CONST_SHAPES[k]) for k in F32_ORDER)


def build_nc():
    Alu = mybir.AluOpType
    Act = mybir.ActivationFunctionType
    nc = bacc.Bacc("TRN2", target_bir_lowering=False, debug=False,
                   num_devices=N_CORES)
    x_d = nc.dram_tensor("x", [B_PER_CORE, 512, 512, 3], DT,
                         kind="ExternalInput").ap()
    o_d = nc.dram_tensor("out", [B_PER_CORE, 512, 512, 3], DT_U8,
                         kind="ExternalOutput").ap()
    ch16_d = nc.dram_tensor("ch16", [128, H16_TOTAL], DT_H,
                            kind="ExternalInput").ap()
    cf32_d = nc.dram_tensor("cf32", [128, F32_TOTAL], DT,
                            kind="ExternalInput").ap()

    with tile.TileContext(nc) as tc:
        with (
            tc.tile_pool(name="cpool", bufs=1) as cpool,
            tc.tile_pool(name="iopool", bufs=6) as iopool,
            tc.tile_pool(name="u8pool", bufs=9) as u8pool,
            tc.tile_pool(name="m1pool", bufs=9) as m1pool,
            tc.tile_pool(name="qpool", bufs=6) as qpool,
            tc.tile_pool(name="m2pool", bufs=9) as m2pool,
            tc.tile_pool(name="m3pool", bufs=9) as m3pool,
            tc.tile_pool(name="opool", bufs=6) as opool,
            tc.tile_pool(name="pspool", bufs=7, space="PSUM") as pspool,
        ):
            ch16_t = cpool.tile([128, H16_TOTAL], DT_H, tag="ch16", name="ch16")
            cf32_t = cpool.tile([128, F32_TOTAL], DT, tag="cf32", name="cf32")
            # p1 consts first; the rest stream in behind image 0's inputs
            nc.sync.dma_start(out=ch16_t[:, 0:H16A_TOTAL],
                              in_=ch16_d[:, 0:H16A_TOTAL])
            ct = {}
            off = 0
            for k in H16_ORDER:
                n = _ncols(CONST_SHAPES[k])
                ap = ch16_t[:, off:off + n]
                s = CONST_SHAPES[k]
                if len(s) == 3:
                    ap = ap.rearrange("p (a b) -> p a b", a=s[1])
                ct[k] = ap
                off += n
            off = 0
            for k in F32_ORDER:
                n = _ncols(CONST_SHAPES[k])
                ap = cf32_t[:, off:off + n]
                s = CONST_SHAPES[k]
                if len(s) == 3:
                    ap = ap.rearrange("p (a b) -> p a b", a=s[1])
                ct[k] = ap
                off += n

            # Fine-grained software pipeline: each image = 28 chunks
            # (4 per stage); images interleaved with a 7-chunk skew so every
            # engine queue rotates across independent work.
            u8 = {}
            m1 = {}
            m2qy = {}
            m2qc = {}
            m3y = {}
            m3c = {}

            def S1(b, r):
                if r == 0:
                    u8[b] = []
                xin = iopool.tile([128, 512, 3], DT, tag="xin", name="xin")
                nc.sync.dma_start(out=xin[:], in_=x_d[b, 128 * r:128 * (r + 1)])
                if b == 0 and r == 1:
                    # p1 consts right behind the first two input tiles
                    nc.sync.dma_start(out=ch16_t[:, 0:H16A_TOTAL],
                                      in_=ch16_d[:, 0:H16A_TOTAL])
                if b == 0 and r == 3:
                    nc.sync.dma_start(out=ch16_t[:, H16A_TOTAL:],
                                      in_=ch16_d[:, H16A_TOTAL:])
                    nc.sync.dma_start(out=cf32_t[:], in_=cf32_d[:])
                u8t = u8pool.tile([128, 512, 3], DT_H, tag="u8", name="u8t")
                # fp16 write rounds 255x+1535.5 to nearest int (ULP=1 range)
                eng = nc.vector if r < 2 else nc.gpsimd
                eng.tensor_scalar(
                    out=u8t[:], in0=xin[:], scalar1=255.0, scalar2=1535.5,
                    op0=Alu.mult, op1=Alu.add)
                nc.vector.tensor_scalar(
                    out=u8t[:], in0=u8t[:], scalar1=1536.0, scalar2=0.0,
                    op0=Alu.subtract, op1=Alu.add)
                u8[b].append(u8t)

            def P1(b, jc):
                if jc == 0:
                    m1[b] = []
                psA = pspool.tile([128, 2, 256], DT, tag="psab", bufs=2, name="psA")
                psB = pspool.tile([128, 2, 256], DT, tag="psab", bufs=2, name="psB")
                for r in range(4):
                    pst = psA if r < 2 else psB
                    g = r % 2
                    for ch in range(3):
                        stat = u8[b][r][:, 128 * jc:128 * (jc + 1), ch]
                        nc.tensor.matmul(
                            pst[:, g, :], stat, ct[f"mvp1_{ch}"],
                            start=(ch == 0), stop=(ch == 2))
                m1t = m1pool.tile([128, 4, 256], DT_H, tag="m1", name="m1t")
                nc.scalar.copy(m1t[:, 0:2, :], psA[:])
                nc.scalar.copy(m1t[:, 2:4, :], psB[:])
                m1[b].append(m1t)
                if jc == 3:
                    del u8[b]

            def P2Y(b, jc2):
                if jc2 == 0:
                    m2qy[b] = []
                ps2 = pspool.tile([128, 4, 128], DT, tag="psmid", bufs=2, name="ps2")
                nc.tensor.matmul(ps2[:], ct["sp2y"],
                                 m1[b][jc2][:, :, 0:128], start=True, stop=True)
                yq = qpool.tile([128, 4, 128], mybir.dt.int16, tag="qy",
                                name="yq")
                # f32->i16 write rounds (RNE): quantization in one op
                nc.vector.tensor_tensor(
                    out=yq[:], in0=ps2[:], in1=ct["rtaby"], op=Alu.mult)
                qt = m2pool.tile([128, 4, 128], DT_H, tag="m2qy", name="qty")
                nc.vector.tensor_tensor(
                    out=qt[:], in0=yq[:], in1=ct["taby"], op=Alu.mult)
                m2qy[b].append(qt)

            def P2C(b, k):
                chi, t_ = divmod(k, 2)
                if k == 0:
                    m2qc[b] = {0: [], 1: []}
                psc = pspool.tile([128, 2, 128], DT, tag="psmid", bufs=2,
                                  name="psc")
                for half in range(2):
                    r2 = 2 * t_ + half
                    nc.tensor.matmul(
                        psc[64 * half:64 * (half + 1), :, :],
                        ct["sp2c"][:, 0:64],
                        m1[b][r2][:, :, 128 + 64 * chi:192 + 64 * chi],
                        start=True, stop=True)
                yc = qpool.tile([128, 2, 128], mybir.dt.int16, tag="qc",
                                name="yc")
                nc.vector.tensor_tensor(
                    out=yc[:], in0=psc[:], in1=ct["rtabc"], op=Alu.mult)
                qt = m2pool.tile([128, 2, 128], DT_H, tag="m2qc", name="qtc")
                nc.gpsimd.tensor_tensor(
                    out=qt[:], in0=yc[:], in1=ct["tabc"], op=Alu.mult)
                m2qc[b][chi].append(qt)
                if k == 3:
                    del m1[b]

            def P3Y(b, jc3):
                if jc3 == 0:
                    m3y[b] = []
                ps3 = pspool.tile([128, 4, 128], DT, tag="psmid", bufs=2, name="ps3")
                for r3 in range(4):
                    nc.tensor.matmul(
                        ps3[:, r3, :], m2qy[b][r3][:, jc3, :],
                        ct["mvp3y"], start=True, stop=True)
                mt = m3pool.tile([128, 4, 128], DT_H, tag="m3y", name="mty")
                nc.scalar.copy(mt[:], ps3[:])
                m3y[b].append(mt)

            def P3C(b, k):
                chi, jc3 = divmod(k, 2)
                if k == 0:
                    m3c[b] = {0: [], 1: []}
                ps3 = pspool.tile([128, 2, 128], DT, tag="psmid", bufs=2,
                                  name="ps3c")
                for r3 in range(2):
                    # half-res w-IDCT: V.T[kw, j] == D[kw, j//2], so the
                    # 2x w-upsample moves into a stride-0 AP in p4
                    nc.tensor.matmul(
                        ps3[:, r3, :], m2qc[b][chi][r3][:, jc3, :],
                        ct["mvp3y"], start=True, stop=True)
                mt = m3pool.tile([128, 2, 128], DT_H, tag="m3c", name="mtc")
                nc.scalar.copy(mt[:], ps3[:])
                m3c[b][chi].append(mt)
                if k == 3:
                    del m2qy[b], m2qc[b]

            def P4(b, r):
                rc, half = divmod(r, 2)
                psR = pspool.tile([128, 512], DT, tag="ps4", bufs=2, name="psR")
                psG = pspool.tile([128, 512], DT, tag="ps4", bufs=2, name="psG")
                psB4 = pspool.tile([128, 512], DT, tag="ps4", bufs=2, name="psB4")
                my = m3y[b][r][:]
                # nearest w-upsample via stride-0 broadcast moving AP
                mcb = m3c[b][0][rc][:].unsqueeze(3).broadcast_to(
                    [128, 2, 128, 2])
                mcr = m3c[b][1][rc][:].unsqueeze(3).broadcast_to(
                    [128, 2, 128, 2])

                def _acc(ps, terms):
                    for i, (cname, mv) in enumerate(terms):
                        nc.tensor.matmul(ps[:], ct[cname], mv,
                                         start=(i == 0),
                                         stop=(i == len(terms) - 1))
                _acc(psR, [("sp4y", my), (f"sp4c_h{half}_rcr", mcr)])
                _acc(psG, [("sp4y", my), (f"sp4c_h{half}_gcb", mcb),
                           (f"sp4c_h{half}_gcr", mcr)])
                _acc(psB4, [("sp4y", my), (f"sp4c_h{half}_bcb", mcb)])
                ot = opool.tile([128, 512, 3], DT_U8, tag="o", name="ot")
                # float->uint8 write = RNE + saturate: whole post-chain
                nc.scalar.activation(ot[:, :, 0], psR[:], Act.Copy,
                                     bias=0.0, scale=1.0)
                nc.vector.tensor_scalar(
                    out=ot[:, :, 1], in0=psG[:], scalar1=0.0, scalar2=0.0,
                    op0=Alu.add, op1=Alu.add)
                if r < 2:
                    nc.scalar.activation(ot[:, :, 2], psB4[:], Act.Copy,
                                         bias=0.0, scale=1.0)
                else:
                    nc.vector.tensor_scalar(
                        out=ot[:, :, 2], in0=psB4[:], scalar1=0.0,
                        scalar2=0.0, op0=Alu.add, op1=Alu.add)
                nc.sync.dma_start(out=o_d[b, 128 * r:128 * (r + 1)], in_=ot[:])
                if r == 3:
                    del m3y[b], m3c[b]

            chunks = []
            for f in (S1, P1, P2Y, P2C, P3Y, P3C, P4):
                for i in range(4):
                    chunks.append((f, i))
            SKEW = 5
            total = len(chunks) + SKEW * (B_PER_CORE - 1)
            for t in range(total):
                for b in range(B_PER_CORE):
                    ci = t - SKEW * b
                    if 0 <= ci < len(chunks):
                        f, i = chunks[ci]
                        f(b, i)

    nc.compile()
    return nc


_CACHE = {}


def kernel(x: np.ndarray) -> np.ndarray:
    assert x.shape == (32, 512, 512, 3)
    if "nc" not in _CACHE:
        _CACHE["nc"] = build_nc()
        _CACHE["consts"] = pack_consts(build_consts())
    nc = _CACHE["nc"]
    consts = _CACHE["consts"]
    xs = np.ascontiguousarray(x.astype(F))
    in_maps = []
    for i in range(N_CORES):
        m = {"x": xs[B_PER_CORE * i:B_PER_CORE * (i + 1)]}
        m.update(consts)
        in_maps.append(m)
    res = run_bass_kernel_spmd(nc, in_maps, list(range(N_CORES)))
    out = np.concatenate([res.results[i]["out"] for i in range(N_CORES)], axis=0)
    return out.astype(np.float32) / np.float32(255.0)


# revision 17
# speedup vs baseline: 1.0848x; 1.0848x over previous
"""JPEG compression roundtrip kernel for Trainium2 (8 NeuronCores, batch-parallel).

Self-contained: builds constants, shards batch 32 -> 8 cores x 4 images,
runs a Bass/Tile kernel per core, gathers full output.

Pipeline per image (512x512x3 f32 in [0,1)), all matmuls fp16 (1 cyc/row):
  S1  Act: fp16 <- 255*x + 1535.5 (write rounds to int); DVE: -1536 -> u8 exact
  p1  (stat=u8 fp16 chunks, mov=color-scaled DCT fp16)  -> M1 = (A@{Y,Cb,Cr})^T
  p2  (stat=DCT const fp16, mov=M1 fp16)                -> coef PSUM f32
  q   y = coef*(1/t) [DVE]; q = rne(y) -> fp16 exact [DVE]; deq = q*t fp16 [DVE 2x]
  p3  (stat=deq fp16, mov=IDCT const fp16)              -> M3 PSUM -> fp16
  p4  (stat=IDCT+color consts fp16, mov=M3 fp16)        -> R,G,B planes PSUM f32
  S5  single op: float->uint8 convert (HW does RNE+saturate) -> interleaved u8 out
Host: out = u8.astype(f32) / 255.

4:2:0 chroma down/upsample folded into chroma DCT matrices (E = D@P, V = 2E^T);
+-128 level shifts cancel because the DC quant step divides the DC shift.
Dequantized coefficients q*t are integers <= 2046: exact in fp16.
"""
import numpy as np

from concourse import bacc, bass, mybir, tile
from concourse.bass_utils import run_bass_kernel_spmd

F = np.float32
C_RNE = float(np.float32(12582912.0))  # 1.5 * 2**23
N_CORES = 8
B_PER_CORE = 4
DT = mybir.dt.float32
DT_H = mybir.dt.float16
DT_U8 = mybir.dt.uint8
QUALITY = 95

_LUMA = np.array([
    [16, 11, 10, 16, 24, 40, 51, 61],
    [12, 12, 14, 19, 26, 58, 60, 55],
    [14, 13, 16, 24, 40, 57, 69, 56],
    [14, 17, 22, 29, 51, 87, 80, 62],
    [18, 22, 37, 56, 68, 109, 103, 77],
    [24, 35, 55, 64, 81, 104, 113, 92],
    [49, 64, 78, 87, 103, 121, 120, 101],
    [72, 92, 95, 98, 112, 100, 103, 99]], dtype=F)
_CHROMA = np.array([
    [17, 18, 24, 47, 99, 99, 99, 99],
    [18, 21, 26, 66, 99, 99, 99, 99],
    [24, 26, 56, 99, 99, 99, 99, 99],
    [47, 66, 99, 99, 99, 99, 99, 99],
    [99, 99, 99, 99, 99, 99, 99, 99],
    [99, 99, 99, 99, 99, 99, 99, 99],
    [99, 99, 99, 99, 99, 99, 99, 99],
    [99, 99, 99, 99, 99, 99, 99, 99]], dtype=F)


def _qtable(base, quality):
    scale = 5000.0 / quality if quality < 50 else 200.0 - 2.0 * quality
    return np.clip(np.floor((base * scale + 50.0) / 100.0), 1.0, 255.0).astype(F)


def build_consts():
    k = np.arange(8)
    D = np.sqrt(2.0 / 8.0) * np.cos((2 * k[None, :] + 1) * k[:, None] * np.pi / 16.0)
    D[0, :] /= np.sqrt(2.0)
    D = D.astype(F)
    P = np.zeros((8, 16), F)
    for i in range(8):
        P[i, 2 * i] = 0.5
        P[i, 2 * i + 1] = 0.5
    E = (D @ P).astype(F)
    V = (2.0 * E.T).astype(F)
    QL = _qtable(_LUMA, QUALITY)
    QC = _qtable(_CHROMA, QUALITY)
    I16 = np.eye(16, dtype=F)
    I8 = np.eye(8, dtype=F)
    cY = np.array([0.299, 0.587, 0.114], F)
    cCb = np.array([-0.168736, -0.331264, 0.5], F)
    cCr = np.array([0.5, -0.418688, -0.081312], F)

    c = {}
    mv_fy = np.kron(I16, D.T).astype(F)
    mv_fc = np.kron(I8, E.T).astype(F)
    for ch in range(3):
        mv = np.ascontiguousarray(np.concatenate(
            [cY[ch] * mv_fy, cCb[ch] * mv_fc, cCr[ch] * mv_fc], axis=1).astype(F))
        c[f"mvp1_{ch}"] = mv.astype(np.float16)
    c["sp2y"] = mv_fy.astype(np.float16)
    c["sp2c"] = np.ascontiguousarray(
        np.pad(mv_fc, ((0, 0), (0, 64)))).astype(np.float16)
    c["mvp3y"] = np.kron(I16, D).astype(np.float16)
    c["sp4y"] = np.kron(I16, D).astype(np.float16)
    sp4c = np.kron(I16, V).T.astype(F)  # [128 fhc, 256 h]
    wR_cr, wG_cb, wG_cr, wB_cb = 1.402, -0.344136, -0.714136, 1.772
    for h in range(2):
        sl = np.ascontiguousarray(sp4c[:, 128 * h:128 * (h + 1)])
        c[f"sp4c_h{h}_rcr"] = (F(wR_cr) * sl).astype(np.float16)
        c[f"sp4c_h{h}_gcb"] = (F(wG_cb) * sl).astype(np.float16)
        c[f"sp4c_h{h}_gcr"] = (F(wG_cr) * sl).astype(np.float16)
        c[f"sp4c_h{h}_bcb"] = (F(wB_cb) * sl).astype(np.float16)
    tY = np.empty((128, 512), F)
    pp, ff = np.meshgrid(np.arange(128), np.arange(512), indexing="ij")
    tY[:] = QL[ff % 8, pp % 8]
    tC = np.empty((128, 256), F)
    pp, ff = np.meshgrid(np.arange(128), np.arange(256), indexing="ij")
    tC[:] = QC[ff % 8, pp % 8]
    c["taby"] = tY.reshape(128, 4, 128).astype(np.float16)
    c["rtaby"] = (1.0 / tY).astype(F).reshape(128, 4, 128).copy()
    c["tabc"] = tC.reshape(128, 2, 128).astype(np.float16)
    c["rtabc"] = (1.0 / tC).astype(F).reshape(128, 2, 128).copy()
    return c


# fp16 consts packed into one [128, 3328] tensor; fp32 (recip tables) into
# one [128, 768]: two DMAs instead of twenty (HWDGE is exclusive, 625ns each).
H16A_ORDER = ["mvp1_0", "mvp1_1", "mvp1_2"]
H16B_ORDER = ["sp2y", "sp2c", "mvp3y",
              "sp4y",
              "sp4c_h0_rcr", "sp4c_h0_gcb", "sp4c_h0_gcr", "sp4c_h0_bcb",
              "sp4c_h1_rcr", "sp4c_h1_gcb", "sp4c_h1_gcr", "sp4c_h1_bcb",
              "taby", "tabc"]
H16_ORDER = H16A_ORDER + H16B_ORDER
F32_ORDER = ["rtaby", "rtabc"]

CONST_SHAPES = {
    "mvp1_0": (128, 256), "mvp1_1": (128, 256), "mvp1_2": (128, 256),
    "sp2y": (128, 128), "sp2c": (128, 128),
    "mvp3y": (128, 128),
    "sp4y": (128, 128),
    "sp4c_h0_rcr": (128, 128), "sp4c_h0_gcb": (128, 128),
    "sp4c_h0_gcr": (128, 128), "sp4c_h0_bcb": (128, 128),
    "sp4c_h1_rcr": (128, 128), "sp4c_h1_gcb": (128, 128),
    "sp4c_h1_gcr": (128, 128), "sp4c_h1_bcb": (128, 128),
    "taby": (128, 4, 128), "rtaby": (128, 4, 128),
    "tabc": (128, 2, 128), "rtabc": (128, 2, 128),
}


def _ncols(shape):
    n = 1
    for s in shape[1:]:
        n *= s
    return n


def pack_consts(c):
    import numpy as _np
    h16 = _np.concatenate(
        [c[k].reshape(128, -1) for k in H16_ORDER], axis=1).astype(_np.float16)
    f32 = _np.concatenate(
        [c[k].reshape(128, -1) for k in F32_ORDER], axis=1).astype(F)
    return {"ch16": _np.ascontiguousarray(h16), "cf32": _np.ascontiguousarray(f32)}


H16A_TOTAL = sum(_ncols(CONST_SHAPES[k]) for k in H16A_ORDER)
H16_TOTAL = sum(_ncols(CONST_SHAPES[k]) for k in H16_ORDER)
F32_TOTAL = sum(_ncols(CONST_SHAPES[k]) for k in F32_ORDER)


def build_nc():
    Alu = mybir.AluOpType
    Act = mybir.ActivationFunctionType
    nc = bacc.Bacc("TRN2", target_bir_lowering=False, debug=False,
                   num_devices=N_CORES)
    x_d = nc.dram_tensor("x", [B_PER_CORE, 512, 512, 3], DT,
                         kind="ExternalInput").ap()
    o_d = nc.dram_tensor("out", [B_PER_CORE, 512, 512, 3], DT_U8,
                         kind="ExternalOutput").ap()
    ch16_d = nc.dram_tensor("ch16", [128, H16_TOTAL], DT_H,
                            kind="ExternalInput").ap()
    cf32_d = nc.dram_tensor("cf32", [128, F32_TOTAL], DT,
                            kind="ExternalInput").ap()

    with tile.TileContext(nc) as tc:
        with (
            tc.tile_pool(name="cpool", bufs=1) as cpool,
            tc.tile_pool(name="iopool", bufs=6) as iopool,
            tc.tile_pool(name="u8pool", bufs=9) as u8pool,
            tc.tile_pool(name="m1pool", bufs=9) as m1pool,
            tc.tile_pool(name="qpool", bufs=6) as qpool,
            tc.tile_pool(name="m2pool", bufs=9) as m2pool,
            tc.tile_pool(name="m3pool", bufs=9) as m3pool,
            tc.tile_pool(name="opool", bufs=6) as opool,
            tc.tile_pool(name="pspool", bufs=7, space="PSUM") as pspool,
        ):
            ch16_t = cpool.tile([128, H16_TOTAL], DT_H, tag="ch16", name="ch16")
            cf32_t = cpool.tile([128, F32_TOTAL], DT, tag="cf32", name="cf32")
            # p1 consts first; the rest stream in behind image 0's inputs
            nc.sync.dma_start(out=ch16_t[:, 0:H16A_TOTAL],
                              in_=ch16_d[:, 0:H16A_TOTAL])
            ct = {}
            off = 0
            for k in H16_ORDER:
                n = _ncols(CONST_SHAPES[k])
                ap = ch16_t[:, off:off + n]
                s = CONST_SHAPES[k]
                if len(s) == 3:
                    ap = ap.rearrange("p (a b) -> p a b", a=s[1])
                ct[k] = ap
                off += n
            off = 0
            for k in F32_ORDER:
                n = _ncols(CONST_SHAPES[k])
                ap = cf32_t[:, off:off + n]
                s = CONST_SHAPES[k]
                if len(s) == 3:
                    ap = ap.rearrange("p (a b) -> p a b", a=s[1])
                ct[k] = ap
                off += n

            # Fine-grained software pipeline: each image = 28 chunks
            # (4 per stage); images interleaved with a 7-chunk skew so every
            # engine queue rotates across independent work.
            u8 = {}
            m1 = {}
            m2qy = {}
            m2qc = {}
            m3y = {}
            m3c = {}

            def S1(b, r):
                if r == 0:
                    u8[b] = []
                xin = iopool.tile([128, 512, 3], DT, tag="xin", name="xin")
                nc.sync.dma_start(out=xin[:], in_=x_d[b, 128 * r:128 * (r + 1)])
                if b == 0 and r == 1:
                    # p1 consts right behind the first two input tiles
                    nc.sync.dma_start(out=ch16_t[:, 0:H16A_TOTAL],
                                      in_=ch16_d[:, 0:H16A_TOTAL])
                if b == 0 and r == 3:
                    nc.sync.dma_start(out=ch16_t[:, H16A_TOTAL:],
                                      in_=ch16_d[:, H16A_TOTAL:])
                    nc.sync.dma_start(out=cf32_t[:], in_=cf32_d[:])
                u8t = u8pool.tile([128, 512, 3], DT_H, tag="u8", name="u8t")
                # fp16 write rounds 255x+1535.5 to nearest int (ULP=1 range)
                eng = nc.vector if r < 2 else nc.gpsimd
                eng.tensor_scalar(
                    out=u8t[:], in0=xin[:], scalar1=255.0, scalar2=1535.5,
                    op0=Alu.mult, op1=Alu.add)
                nc.vector.tensor_scalar(
                    out=u8t[:], in0=u8t[:], scalar1=1536.0, scalar2=0.0,
                    op0=Alu.subtract, op1=Alu.add)
                u8[b].append(u8t)

            def P1(b, jc):
                if jc == 0:
                    m1[b] = []
                psAB = pspool.tile([128, 4, 256], DT, tag="psab", bufs=2,
                                   name="psAB")
                for r in range(4):
                    for ch in range(3):
                        stat = u8[b][r][:, 128 * jc:128 * (jc + 1), ch]
                        nc.tensor.matmul(
                            psAB[:, r, :], stat, ct[f"mvp1_{ch}"],
                            start=(ch == 0), stop=(ch == 2))
                m1t = m1pool.tile([128, 4, 256], DT_H, tag="m1", name="m1t")
                nc.scalar.copy(m1t[:], psAB[:])
                m1[b].append(m1t)
                if jc == 3:
                    del u8[b]

            def P2Y(b, jc2):
                if jc2 == 0:
                    m2qy[b] = []
                ps2 = pspool.tile([128, 4, 128], DT, tag="psmid", bufs=2, name="ps2")
                nc.tensor.matmul(ps2[:], ct["sp2y"],
                                 m1[b][jc2][:, :, 0:128], start=True, stop=True)
                yq = qpool.tile([128, 4, 128], mybir.dt.int16, tag="qy",
                                name="yq")
                # f32->i16 write rounds (RNE): quantization in one op
                nc.vector.tensor_tensor(
                    out=yq[:], in0=ps2[:], in1=ct["rtaby"], op=Alu.mult)
                qt = m2pool.tile([128, 4, 128], DT_H, tag="m2qy", name="qty")
                nc.vector.tensor_tensor(
                    out=qt[:], in0=yq[:], in1=ct["taby"], op=Alu.mult)
                m2qy[b].append(qt)

            def P2C(b, k):
                chi, t_ = divmod(k, 2)
                if k == 0:
                    m2qc[b] = {0: [], 1: []}
                psc = pspool.tile([128, 2, 128], DT, tag="psmid", bufs=2,
                                  name="psc")
                for half in range(2):
                    r2 = 2 * t_ + half
                    nc.tensor.matmul(
                        psc[64 * half:64 * (half + 1), :, :],
                        ct["sp2c"][:, 0:64],
                        m1[b][r2][:, :, 128 + 64 * chi:192 + 64 * chi],
                        start=True, stop=True)
                yc = qpool.tile([128, 2, 128], mybir.dt.int16, tag="qc",
                                name="yc")
                nc.vector.tensor_tensor(
                    out=yc[:], in0=psc[:], in1=ct["rtabc"], op=Alu.mult)
                qt = m2pool.tile([128, 2, 128], DT_H, tag="m2qc", name="qtc")
                nc.gpsimd.tensor_tensor(
                    out=qt[:], in0=yc[:], in1=ct["tabc"], op=Alu.mult)
                m2qc[b][chi].append(qt)
                if k == 3:
                    del m1[b]

            def P3Y(b, jc3):
                if jc3 == 0:
                    m3y[b] = []
                ps3 = pspool.tile([128, 4, 128], DT, tag="psmid", bufs=2, name="ps3")
                for r3 in range(4):
                    nc.tensor.matmul(
                        ps3[:, r3, :], m2qy[b][r3][:, jc3, :],
                        ct["mvp3y"], start=True, stop=True)
                mt = m3pool.tile([128, 4, 128], DT_H, tag="m3y", name="mty")
                nc.scalar.copy(mt[:], ps3[:])
                m3y[b].append(mt)

            def P3C(b, k):
                chi, jc3 = divmod(k, 2)
                if k == 0:
                    m3c[b] = {0: [], 1: []}
                ps3 = pspool.tile([128, 2, 128], DT, tag="psmid", bufs=2,
                                  name="ps3c")
                for r3 in range(2):
                    # half-res w-IDCT: V.T[kw, j] == D[kw, j//2], so the
                    # 2x w-upsample moves into a stride-0 AP in p4
                    nc.tensor.matmul(
                        ps3[:, r3, :], m2qc[b][chi][r3][:, jc3, :],
                        ct["mvp3y"], start=True, stop=True)
                mt = m3pool.tile([128, 2, 128], DT_H, tag="m3c", name="mtc")
                nc.scalar.copy(mt[:], ps3[:])
                m3c[b][chi].append(mt)
                if k == 3:
                    del m2qy[b], m2qc[b]

            def P4(b, r):
                rc, half = divmod(r, 2)
                psR = pspool.tile([128, 512], DT, tag="ps4", bufs=2, name="psR")
                psG = pspool.tile([128, 512], DT, tag="ps4", bufs=2, name="psG")
                psB4 = pspool.tile([128, 512], DT, tag="ps4", bufs=2, name="psB4")
                my = m3y[b][r][:]
                # nearest w-upsample via stride-0 broadcast moving AP
                mcb = m3c[b][0][rc][:].unsqueeze(3).broadcast_to(
                    [128, 2, 128, 2])
                mcr = m3c[b][1][rc][:].unsqueeze(3).broadcast_to(
                    [128, 2, 128, 2])

                def _acc(ps, terms):
                    for i, (cname, mv) in enumerate(terms):
                        nc.tensor.matmul(ps[:], ct[cname], mv,
                                         start=(i == 0),
                                         stop=(i == len(terms) - 1))
                _acc(psR, [("sp4y", my), (f"sp4c_h{half}_rcr", mcr)])
                _acc(psG, [("sp4y", my), (f"sp4c_h{half}_gcb", mcb),
                           (f"sp4c_h{half}_gcr", mcr)])
                _acc(psB4, [("sp4y", my), (f"sp4c_h{half}_bcb", mcb)])
                ot = opool.tile([128, 512, 3], DT_U8, tag="o", name="ot")
                # float->uint8 write = RNE + saturate: whole post-chain
                nc.scalar.activation(ot[:, :, 0], psR[:], Act.Copy,
                                     bias=0.0, scale=1.0)
                nc.vector.tensor_scalar(
                    out=ot[:, :, 1], in0=psG[:], scalar1=0.0, scalar2=0.0,
                    op0=Alu.add, op1=Alu.add)
                if r < 2:
                    nc.scalar.activation(ot[:, :, 2], psB4[:], Act.Copy,
                                         bias=0.0, scale=1.0)
                else:
                    nc.vector.tensor_scalar(
                        out=ot[:, :, 2], in0=psB4[:], scalar1=0.0,
                        scalar2=0.0, op0=Alu.add, op1=Alu.add)
                nc.sync.dma_start(out=o_d[b, 128 * r:128 * (r + 1)], in_=ot[:])
                if r == 3:
                    del m3y[b], m3c[b]

            STAGES = [(S1, 0), (P1, 1), (P2C, 2), (P2Y, 2), (P3C, 3),
                      (P3Y, 3), (P4, 4)]
            for step in range(B_PER_CORE + 5):
                for f, off in reversed(STAGES):
                    b = step - off
                    if 0 <= b < B_PER_CORE:
                        for i in range(4):
                            f(b, i)

    nc.compile()
    return nc


_CACHE = {}


def kernel(x: np.ndarray) -> np.ndarray:
    assert x.shape == (32, 512, 512, 3)
    if "nc" not in _CACHE:
        _CACHE["nc"] = build_nc()
        _CACHE["consts"] = pack_consts(build_consts())
    nc = _CACHE["nc"]
    consts = _CACHE["consts"]
    xs = np.ascontiguousarray(x.astype(F))
    in_maps = []
    for i in range(N_CORES):
        m = {"x": xs[B_PER_CORE * i:B_PER_CORE * (i + 1)]}
        m.update(consts)
        in_maps.append(m)
    res = run_bass_kernel_spmd(nc, in_maps, list(range(N_CORES)))
    out = np.concatenate([res.results[i]["out"] for i in range(N_CORES)], axis=0)
    return out.astype(np.float32) / np.float32(255.0)
